# revision 11
# baseline (speedup 1.0000x reference)
"""Trainium2 Bass kernel for AdaptiveTopKMoE (N=8192, H=2048, E=8, K=2, CF=1.25).

Expert-parallel across 8 NeuronCores: core i owns expert i's W1/W2 shard and
token shard [1024*i, 1024*(i+1)). Routing is computed replicated on every core
from an AllGathered logits tensor; capacity ranks come from exclusive prefix
sums done with triangular-ones matmuls; the per-expert compact table is built
with one-hot matmuls on the PE.

v2 layout: weights are pre-packed on the host into per-output-tile bf16 blocks
([128, contract*out] contiguous) so weight streaming runs on the HWDGE (sync/
scalar) engines as contiguous line-rate DMAs instead of 2048-descriptor
strided gpsimd DMAs. Tokens are also shipped pre-converted to bf16 so the FFN
x gather is a single transposed dma_gather straight into the [contract,
token] layout (no PE transposes / vector converts on the gather path). The
FFN runs in one 1280-slot pass (not two 640 halves), halving weight traffic.
The combine is 4 H-chunk AllGathers of gate-weighted expert outputs, each
overlapped with the next chunk's GEMM2, plus an owner-side dma_gather.
Invalid/dropped dispatches resolve to a zeroed trash row (slot 1280) so the
combine needs no masking.
"""

import os
import sys

for _p in ("/opt/trn_rl_repo", "/root/.axon_site/_ro/trn_rl_repo", "/root/.axon_site"):
    if os.path.isdir(_p) and _p not in sys.path:
        sys.path.insert(0, _p)

import numpy as np
import ml_dtypes

from concourse import bass, bacc, mybir, tile
from concourse.bass import ds
from concourse.bass_utils import run_bass_kernel_spmd

F32 = mybir.dt.float32
BF16 = mybir.dt.bfloat16
I32 = mybir.dt.int32
I16 = mybir.dt.int16
AF = mybir.ActivationFunctionType
AO = mybir.AluOpType
BF_NP = ml_dtypes.bfloat16

N, H, F, E, K = 8192, 2048, 4096, 8, 2
CAP = 1280            # ceil(1.25 * N / E)
NC = 8                # cores
TSH = N // NC         # 1024 tokens per core shard
RH = CAP + 1          # 1281 rows per core AG input (row 1280 = zero trash)
AGB = NC * RH         # 10248 rows in each AG output
HC = 512              # H-chunk per y AllGather
NHB = H // HC         # 4 H-chunks
NCH = [(0, 512), (512, 512), (1024, 256)]  # token n-chunks within the 1280 pass
NSL = CAP // 128      # 10 slot blocks of 128

LAST_EXEC_NS = None

# Sim-only escape hatch: CoreSim locks each SWDGE sem to one queue, so the
# round-robin queue_num choices below trip it. KSIMQ=1 forces queue 0.
_SINGLE_Q = bool(int(os.environ.get("KSIMQ", "0")))


def _qn(i):
    return 0 if _SINGLE_Q else i % 4


def build_nc(debug=False, do_compile=True, stage=5):
    nc = bacc.Bacc(
        "TRN2", target_bir_lowering=False, debug=False, num_devices=NC,
        num_swdge_queues=4,
    )

    tok_sh = nc.dram_tensor("tok_sh", [TSH, H], F32, kind="ExternalInput").ap()
    tokens_bf = nc.dram_tensor("tokens_bf", [N, H], BF16, kind="ExternalInput").ap()
    Wr = nc.dram_tensor("Wr", [E, H], F32, kind="ExternalInput").ap()
    br = nc.dram_tensor("br", [1, E], F32, kind="ExternalInput").ap()
    W1b = nc.dram_tensor("W1b", [F // 128, 128, H], BF16, kind="ExternalInput").ap()
    b1s = nc.dram_tensor("b1s", [1, F], F32, kind="ExternalInput").ap()
    W2b = nc.dram_tensor("W2b", [H // 128, 128, F], BF16, kind="ExternalInput").ap()
    b2s = nc.dram_tensor("b2s", [1, H], F32, kind="ExternalInput").ap()
    out = nc.dram_tensor("out", [TSH, H], F32, kind="ExternalOutput").ap()

    lg_slice = nc.dram_tensor("lg_slice", [TSH, E], F32).ap()
    lg_full = nc.dram_tensor("lg_full", [N, E], F32, addr_space="Shared").ap()
    g_lin = nc.dram_tensor("g_lin", [2 * N], F32).ap()
    tok_dram = nc.dram_tensor("tok_dram", [CAP], I16).ap()
    # y_w[hc]: this core's gate-weighted expert output rows for one H-chunk
    y_w = [nc.dram_tensor(f"y_w{b}", [RH, HC], BF16).ap() for b in range(NHB)]
    # y_ag[hc]: AllGathered blocks, row e*RH + s
    y_ag = [
        nc.dram_tensor(f"y_ag{b}", [AGB, HC], BF16, addr_space="Shared").ap()
        for b in range(NHB)
    ]
    if debug:
        dbg_rk = nc.dram_tensor("dbg_rk", [4, N], F32, kind="ExternalOutput").ap()

    rg = [list(range(NC))]

    with tile.TileContext(nc) as tc:
        with (
            tc.tile_pool(name="main", bufs=1) as pm,
            tc.tile_pool(name="ppA", bufs=3, space="PSUM") as ppA,
            tc.tile_pool(name="ppB", bufs=2, space="PSUM") as ppB,
        ):
            # ---------------- phase 0: constants ----------------
            pid_gp = nc.gpsimd.partition_id()

            iotaP = pm.tile([128, 1], I32, tag="iotaP")
            nc.gpsimd.iota(iotaP[:], pattern=[[0, 1]], base=0, channel_multiplier=1)
            pcol = pm.tile([128, 1], F32, tag="pcol")
            nc.vector.tensor_copy(pcol[:], iotaP[:])

            iotaF = pm.tile([128, 128], I32, tag="iotaF")
            nc.gpsimd.iota(iotaF[:], pattern=[[1, 128]], base=0, channel_multiplier=0)
            frow = pm.tile([128, 128], F32, tag="frow")
            nc.vector.tensor_copy(frow[:], iotaF[:])

            U = pm.tile([128, 128], F32, tag="U")
            nc.vector.tensor_scalar(U[:], frow[:], pcol[:], None, op0=AO.is_gt)
            ident = pm.tile([128, 128], F32, tag="ident")
            nc.vector.tensor_scalar(ident[:], frow[:], pcol[:], None, op0=AO.is_equal)
            identb = pm.tile([128, 128], BF16, tag="identb")
            nc.vector.tensor_copy(identb[:], ident[:])

            iotaE3 = pm.tile([128, 64, 8], I32, tag="iotaE3")
            nc.gpsimd.iota(iotaE3[:], pattern=[[0, 64], [1, 8]], base=0, channel_multiplier=0)
            iotaE3f = pm.tile([128, 64, 8], F32, tag="iotaE3f")
            nc.vector.tensor_copy(iotaE3f[:], iotaE3[:])

            tio = pm.tile([128, 64], I32, tag="tio")
            nc.gpsimd.iota(tio[:], pattern=[[128, 64]], base=0, channel_multiplier=1)
            tiof = pm.tile([128, 64], F32, tag="tiof")
            nc.vector.tensor_copy(tiof[:], tio[:])

            b1sb = pm.tile([128, F // 128], F32, tag="b1sb")
            nc.sync.dma_start(out=b1sb[:], in_=b1s.rearrange("o (m p) -> p (o m)", p=128))
            b2sb = pm.tile([128, H // 128], F32, tag="b2sb")
            nc.sync.dma_start(out=b2sb[:], in_=b2s.rearrange("o (m p) -> p (o m)", p=128))

            WrT = pm.tile([128, H // 128, E], F32, tag="WrT")
            WrT_src = Wr.rearrange("e (kt p) -> p kt e", p=128)
            for kt in range(H // 128):
                nc.sync.dma_start(out=WrT[:, kt, :], in_=WrT_src[:, kt, :])

            brsb = pm.tile([1, E], F32, tag="brsb")
            nc.sync.dma_start(out=brsb[:], in_=br[:, :])
            brb = pm.tile([128, E], F32, tag="brb")
            nc.gpsimd.partition_broadcast(brb[:], brsb[:])

            pidt = pm.tile([1, 1], I32, tag="pidt")
            nc.gpsimd.reg_save(pidt[0:1, 0:1], pid_gp)
            pidb = pm.tile([128, 1], I32, tag="pidb")
            nc.gpsimd.partition_broadcast(pidb[:], pidt[:])
            pidf = pm.tile([128, 1], F32, tag="pidf")
            nc.vector.tensor_copy(pidf[:], pidb[:])

            zrow = pm.tile([1, HC], BF16, tag="zrow")
            nc.vector.memset(zrow[:], 0.0)
            # trash rows of the 4 AG inputs: write once, up front
            for b in range(NHB):
                nc.scalar.dma_start(out=y_w[b][CAP : CAP + 1, :], in_=zrow[:])
            # one-hot table-build constants: s128b[p, tau, S] = S*128
            s128b = pm.tile([128, 64, NSL], F32, tag="s128b")
            s128i = pm.tile([128, 64, NSL], I32, tag="s128i")
            nc.gpsimd.iota(s128i[:], pattern=[[0, 64], [128, NSL]], base=0, channel_multiplier=0)
            nc.vector.tensor_copy(s128b[:], s128i[:])
            w10 = pm.tile([128, NSL], F32, tag="w10")

            tokidx = pm.tile([128, CAP // 16], I16, tag="tokidx")
            ownidx = pm.tile([128, 2 * TSH // 16], I16, tag="ownidx")

            # ---------------- phases 1-2: router + routing + tables ----------
            with tc.tile_pool(name="route", bufs=1) as pr, tc.tile_pool(
                name="rtok", bufs=3
            ) as prt:
                for tt in range(TSH // 128):
                    tsl = prt.tile([128, H], F32, tag="tsl")
                    teng = nc.sync if tt % 2 == 0 else nc.scalar
                    teng.dma_start(out=tsl[:], in_=tok_sh[tt * 128 : (tt + 1) * 128, :])
                    xTr = prt.tile([128, H // 128, 128], F32, tag="xTr")
                    for kt in range(H // 128):
                        ps = ppB.tile([128, 128], F32, tag="tp")
                        nc.tensor.transpose(ps[:], tsl[:, kt * 128 : (kt + 1) * 128], ident[:])
                        # split PSUM->SBUF copies across scalar+vector so the
                        # router head isn't serialized on one engine
                        if kt % 2 == 0:
                            nc.scalar.activation(xTr[:, kt, :], ps[:], AF.Copy)
                        else:
                            nc.vector.tensor_copy(xTr[:, kt, :], ps[:])
                    plg = ppB.tile([128, E], F32, tag="plg")
                    for kt in range(H // 128):
                        nc.tensor.matmul(
                            plg[:], xTr[:, kt, :], WrT[:, kt, :],
                            start=(kt == 0), stop=(kt == H // 128 - 1),
                        )
                    lgs = pr.tile([128, E], F32, tag="lgs")
                    nc.vector.tensor_tensor(lgs[:], plg[:], brb[:], op=AO.add)
                    nc.sync.dma_start(out=lg_slice[tt * 128 : (tt + 1) * 128, :], in_=lgs[:])

                nc.gpsimd.collective_compute(
                    "AllGather", AO.bypass, replica_groups=rg,
                    ins=[lg_slice[:, :]], outs=[lg_full[:, :]],
                )

                lg = pr.tile([128, 64, 8], F32, tag="lg")
                nc.sync.dma_start(out=lg[:], in_=lg_full.rearrange("(tau p) e -> p tau e", p=128))
                m1 = pr.tile([128, 64], F32, tag="m1")
                nc.vector.tensor_reduce(m1[:], lg[:], axis=mybir.AxisListType.X, op=AO.max)
                eq1 = pr.tile([128, 64, 8], F32, tag="eq1")
                nc.vector.tensor_tensor(
                    eq1[:], lg[:], m1[:].to_broadcast((128, 64, 8)), op=AO.is_equal
                )
                tmp3 = pr.tile([128, 64, 8], F32, tag="tmp3")
                nc.vector.tensor_tensor(tmp3[:], eq1[:], iotaE3f[:], op=AO.mult)
                idx1 = pr.tile([128, 64], F32, tag="idx1")
                nc.vector.tensor_reduce(idx1[:], tmp3[:], axis=mybir.AxisListType.X, op=AO.add)
                masked = pr.tile([128, 64, 8], F32, tag="masked")
                nc.vector.scalar_tensor_tensor(
                    masked[:], eq1[:], -1e30, lg[:], op0=AO.mult, op1=AO.add
                )
                m2 = pr.tile([128, 64], F32, tag="m2")
                nc.vector.tensor_reduce(m2[:], masked[:], axis=mybir.AxisListType.X, op=AO.max)
                eq2 = pr.tile([128, 64, 8], F32, tag="eq2")
                nc.vector.tensor_tensor(
                    eq2[:], masked[:], m2[:].to_broadcast((128, 64, 8)), op=AO.is_equal
                )
                nc.vector.tensor_tensor(tmp3[:], eq2[:], iotaE3f[:], op=AO.mult)
                idx2 = pr.tile([128, 64], F32, tag="idx2")
                nc.vector.tensor_reduce(idx2[:], tmp3[:], axis=mybir.AxisListType.X, op=AO.add)
                dd = pr.tile([128, 64], F32, tag="dd")
                nc.vector.tensor_tensor(dd[:], m2[:], m1[:], op=AO.subtract)
                ex = pr.tile([128, 64], F32, tag="ex")
                nc.scalar.activation(ex[:], dd[:], AF.Exp)
                exp1 = pr.tile([128, 64], F32, tag="exp1")
                nc.vector.tensor_scalar_add(exp1[:], ex[:], 1.0)
                p1 = pr.tile([128, 64], F32, tag="p1")
                nc.vector.reciprocal(p1[:], exp1[:])
                p2 = pr.tile([128, 64], F32, tag="p2")
                nc.vector.tensor_scalar(p2[:], p1[:], -1.0, 1.0, op0=AO.mult, op1=AO.add)

                C = pr.tile([128, 64, 8], F32, tag="C")
                nc.vector.tensor_tensor(C[:], eq1[:], eq2[:], op=AO.add)
                C512 = C[:].rearrange("p a e -> p (a e)")
                Pin = ppA.tile([128, 512], F32, tag="acc")
                nc.tensor.matmul(Pin[:], U[:], C512, start=True, stop=True)
                ones1 = pr.tile([128, 1], F32, tag="ones1")
                nc.vector.memset(ones1[:], 1.0)
                Tps = ppA.tile([128, 512], F32, tag="acc")
                nc.tensor.matmul(Tps[0:1, :], ones1[:], C512, start=True, stop=True)
                Trow = pr.tile([1, 512], F32, tag="Trow")
                nc.scalar.activation(Trow[:], Tps[0:1, :], AF.Copy)
                Tcol = pr.tile([64, 8], F32, tag="Tcol")
                nc.sync.dma_start(out=Tcol[:], in_=Trow[:])
                cps = ppB.tile([64, 8], F32, tag="plg")
                nc.tensor.matmul(cps[:], U[0:64, 0:64], Tcol[:], start=True, stop=True)
                csb = pr.tile([64, 8], F32, tag="csb")
                nc.scalar.activation(csb[:], cps[:], AF.Copy)
                crow = pr.tile([1, 512], F32, tag="crow")
                nc.sync.dma_start(out=crow[:], in_=csb[:])
                cb = pr.tile([128, 512], F32, tag="cb")
                nc.gpsimd.partition_broadcast(cb[:], crow[:])
                rkf = pr.tile([128, 64, 8], F32, tag="rkf")
                nc.vector.tensor_tensor(
                    rkf[:].rearrange("p a e -> p (a e)"), Pin[:], cb[:], op=AO.add
                )
                rank1 = pr.tile([128, 64], F32, tag="rank1")
                nc.vector.tensor_tensor(tmp3[:], rkf[:], eq1[:], op=AO.mult)
                nc.vector.tensor_reduce(rank1[:], tmp3[:], axis=mybir.AxisListType.X, op=AO.add)
                rank2 = pr.tile([128, 64], F32, tag="rank2")
                nc.vector.tensor_tensor(tmp3[:], rkf[:], eq2[:], op=AO.mult)
                nc.vector.tensor_reduce(rank2[:], tmp3[:], axis=mybir.AxisListType.X, op=AO.add)
                valid1 = pr.tile([128, 64], F32, tag="valid1")
                nc.vector.tensor_scalar(valid1[:], rank1[:], float(CAP), None, op0=AO.is_lt)
                valid2 = pr.tile([128, 64], F32, tag="valid2")
                nc.vector.tensor_scalar(valid2[:], rank2[:], float(CAP), None, op0=AO.is_lt)
                w1t_ = pr.tile([128, 64], F32, tag="w1t_")
                nc.vector.tensor_tensor(w1t_[:], p1[:], valid1[:], op=AO.mult)
                w2t_ = pr.tile([128, 64], F32, tag="w2t_")
                nc.vector.tensor_tensor(w2t_[:], p2[:], valid2[:], op=AO.mult)
                # rp_k = valid ? rank : CAP
                rp1 = pr.tile([128, 64], F32, tag="rp1")
                nc.vector.scalar_tensor_tensor(
                    rp1[:], rank1[:], -float(CAP), valid1[:], op0=AO.add, op1=AO.mult
                )
                nc.vector.tensor_scalar_add(rp1[:], rp1[:], float(CAP))
                rp2 = pr.tile([128, 64], F32, tag="rp2")
                nc.vector.scalar_tensor_tensor(
                    rp2[:], rank2[:], -float(CAP), valid2[:], op0=AO.add, op1=AO.mult
                )
                nc.vector.tensor_scalar_add(rp2[:], rp2[:], float(CAP))
                # AG row ids: g = idx*RH + rp; invalid (rp = CAP) lands on the
                # expert's trash row automatically.
                g1 = pr.tile([128, 64], F32, tag="g1")
                g2 = pr.tile([128, 64], F32, tag="g2")
                for (gk, idxk, rpk) in ((g1, idx1, rp1), (g2, idx2, rp2)):
                    nc.vector.scalar_tensor_tensor(
                        gk[:], idxk[:], float(RH), rpk[:], op0=AO.mult, op1=AO.add
                    )
                if debug:
                    dview = dbg_rk.rearrange("k (tau p) -> k p tau", p=128)
                    nc.sync.dma_start(out=dview[0], in_=rank1[:])
                    nc.sync.dma_start(out=dview[1], in_=rank2[:])
                    nc.sync.dma_start(out=dview[2], in_=g1[:])
                    nc.sync.dma_start(out=dview[3], in_=g2[:])

                # expert-side select for my expert
                m1e = pr.tile([128, 64], F32, tag="m1e")
                nc.vector.tensor_scalar(m1e[:], idx1[:], pidf[:], None, op0=AO.is_equal)
                m2e = pr.tile([128, 64], F32, tag="m2e")
                nc.vector.tensor_scalar(m2e[:], idx2[:], pidf[:], None, op0=AO.is_equal)
                w_e = pr.tile([128, 64], F32, tag="w_e")
                nc.vector.tensor_tensor(w_e[:], w1t_[:], m1e[:], op=AO.mult)
                tmpb = pr.tile([128, 64], F32, tag="tmpb")
                nc.vector.tensor_tensor(tmpb[:], w2t_[:], m2e[:], op=AO.mult)
                nc.vector.tensor_tensor(w_e[:], w_e[:], tmpb[:], op=AO.add)
                # local slot: m1e*(rp1-CAP) + m2e*(rp2-CAP) + CAP
                sl = pr.tile([128, 64], F32, tag="sl")
                nc.vector.tensor_scalar_add(tmpb[:], rp1[:], -float(CAP))
                nc.vector.tensor_tensor(sl[:], tmpb[:], m1e[:], op=AO.mult)
                nc.vector.tensor_scalar_add(tmpb[:], rp2[:], -float(CAP))
                nc.vector.tensor_tensor(tmpb[:], tmpb[:], m2e[:], op=AO.mult)
                nc.vector.tensor_tensor(sl[:], sl[:], tmpb[:], op=AO.add)
                nc.vector.tensor_scalar_add(sl[:], sl[:], float(CAP))

                if stage >= 2:
                    # table build via one-hot matmuls on the (idle) PE:
                    # slot = S*128 + r; psum[r, (S, c)] += 1{r(t)=r} * qm_S(t) * val_c(t)
                    # accumulated over the 64 token-chunks. Invalid slots
                    # (sl=CAP=1280) have qm=0 for all S -> contribute nothing.
                    # t1 = S*128 - sl (bulk), qm = 1{-128 < t1 <= 0}
                    t1b = pr.tile([128, 64, NSL], F32, tag="t1b")
                    nc.vector.tensor_tensor(
                        t1b[:], s128b[:], sl[:].to_broadcast((128, 64, NSL)),
                        op=AO.subtract,
                    )
                    qma = pr.tile([128, 64, NSL], F32, tag="qma")
                    nc.vector.tensor_scalar(qma[:], t1b[:], -128.0, None, op0=AO.is_gt)
                    qmb = pr.tile([128, 64, NSL], F32, tag="qmb")
                    nc.vector.tensor_scalar(qmb[:], t1b[:], 0.5, None, op0=AO.is_lt)
                    nc.vector.tensor_tensor(qma[:], qma[:], qmb[:], op=AO.mult)
                    # r = sl mod 128 = -sum_S qm*t1
                    nc.vector.tensor_tensor(qmb[:], qma[:], t1b[:], op=AO.mult)
                    rmod = pr.tile([128, 64], F32, tag="rmod")
                    nc.vector.tensor_reduce(rmod[:], qmb[:], axis=mybir.AxisListType.X, op=AO.add)
                    nc.vector.tensor_scalar(rmod[:], rmod[:], -1.0, None, op0=AO.mult)

                    # bulk-precompute both one-hot payload planes in 2 DVE ops
                    # (instead of 128 small ops inside the tau loop)
                    valqAB = pr.tile([128, 64, 2, NSL], F32, tag="valqAB")
                    nc.vector.tensor_tensor(
                        valqAB[:, :, 0, :], qma[:], tiof[:].to_broadcast((128, 64, NSL)),
                        op=AO.mult,
                    )
                    nc.vector.tensor_tensor(
                        valqAB[:, :, 1, :], qma[:], w_e[:].to_broadcast((128, 64, NSL)),
                        op=AO.mult,
                    )
                    with tc.tile_pool(name="ptab", bufs=1, space="PSUM") as ppt, \
                         tc.tile_pool(name="onehot", bufs=3) as poh:
                        pstab = ppt.tile([128, 2, NSL], F32, tag="ptab")
                        pstab2 = pstab[:].rearrange("p c a -> p (c a)")
                        for tau in range(64):
                            R = poh.tile([128, 128], F32, tag="R")
                            nc.vector.tensor_scalar(
                                R[:], frow[:], rmod[:, tau : tau + 1], None,
                                op0=AO.is_equal,
                            )
                            nc.tensor.matmul(
                                pstab2, R[:],
                                valqAB[:, tau, :, :].rearrange("p c a -> p (c a)"),
                                start=(tau == 0), stop=(tau == 63),
                            )
                        # weights stay on-chip in FFN layout [p, S]
                        nc.scalar.activation(w10[:], pstab[:, 1, :], AF.Copy)
                        # token ids -> DRAM (slot order) -> 16-wrap -> doublings
                        toki = pr.tile([128, NSL], I16, tag="toki")
                        nc.vector.tensor_copy(toki[:], pstab[:, 0, :])
                    nc.sync.dma_start(
                        out=tok_dram.rearrange("(s p) -> p s", p=128), in_=toki[:]
                    )
                    nc.sync.dma_start(
                        out=tokidx[0:16, :],
                        in_=tok_dram.rearrange("(f part) -> part f", part=16),
                    )
                    nc.scalar.dma_start(out=tokidx[16:32, :], in_=tokidx[0:16, :])
                    nc.gpsimd.dma_start(out=tokidx[32:64, :], in_=tokidx[0:32, :])
                    nc.sync.dma_start(out=tokidx[64:128, :], in_=tokidx[0:64, :])

                    # owner-side gather ids j = k*TSH + t_local, wrapped in 16
                    nc.sync.dma_start(out=g_lin[0:N].rearrange("(tau p) -> p tau", p=128), in_=g1[:])
                    nc.sync.dma_start(
                        out=g_lin[N : 2 * N].rearrange("(tau p) -> p tau", p=128), in_=g2[:]
                    )
                    ownf = pr.tile([16, 2 * TSH // 16], F32, tag="ownf")
                    gview = g_lin.rearrange("(k o f part) -> part k o f", part=16, k=2, o=8)
                    pid_sy = nc.sync.partition_id()
                    pid_sc = nc.scalar.partition_id()
                    for k, (eng, pid_e) in enumerate(
                        ((nc.sync, pid_sy), (nc.scalar, pid_sc))
                    ):
                        eng.dma_start(
                            out=ownf[:, k * 64 : (k + 1) * 64],
                            in_=gview[:, k, ds(pid_e, 1), :],
                        )
                    nc.vector.tensor_copy(ownidx[0:16, :], ownf[:])
                    nc.scalar.dma_start(out=ownidx[16:32, :], in_=ownidx[0:16, :])
                    nc.gpsimd.dma_start(out=ownidx[32:64, :], in_=ownidx[0:32, :])
                    nc.sync.dma_start(out=ownidx[64:128, :], in_=ownidx[0:64, :])

            # ---------------- phases 3-5: gather, GEMM1, GEMM2+AG ----------
            if stage >= 3:
                with (
                    tc.tile_pool(name="ffn", bufs=1) as pf,
                    tc.tile_pool(name="w1p", bufs=3) as pw1,
                    tc.tile_pool(name="w2p", bufs=2) as pw2,
                    tc.tile_pool(name="stg", bufs=2) as pst,
                ):
                    # chunked transposed gathers (SWDGE descriptor FIFO holds
                    # 128 entries; a 1280-idx transposed gather needs 259):
                    # xTc[ci][p, kt, s] = tokens_bf[tok[ns+s], kt*128+p]
                    xT0 = pf.tile([128, H // 128, NCH[0][1]], BF16, tag="xT0")
                    xT1 = pf.tile([128, H // 128, NCH[1][1]], BF16, tag="xT1")
                    xT2 = pf.tile([128, H // 128, NCH[2][1]], BF16, tag="xT2")
                    xTc = [xT0, xT1, xT2]
                    h1T = pf.tile([128, F // 128, CAP], BF16, tag="h1T")
                    for ci, (ns, nl) in enumerate(NCH):
                        nc.gpsimd.dma_gather(
                            xTc[ci][:, :, :],
                            tokens_bf[:, :],
                            tokidx[:, ns // 16 : (ns + nl) // 16],
                            nl,
                            nl,
                            H,
                            transpose=True,
                            queue_num=_qn(ci),
                        )
                    # GEMM1 + gelu
                    for mf in range(F // 128):
                        w1t = pw1.tile([128, H // 128, 128], BF16, tag="w1")
                        eng = nc.sync if mf % 2 == 0 else nc.scalar
                        eng.dma_start(
                            out=w1t[:],
                            in_=W1b[mf].rearrange("p (kt f) -> p kt f", kt=H // 128),
                        )
                        for ci, (ns, nl) in enumerate(NCH):
                            ps = ppA.tile([128, 512], F32, tag="acc")
                            for kt in range(H // 128):
                                nc.tensor.matmul(
                                    ps[:, :nl],
                                    w1t[:, kt, :],
                                    xTc[ci][:, kt, :],
                                    start=(kt == 0),
                                    stop=(kt == H // 128 - 1),
                                )
                            nc.scalar.activation(
                                h1T[:, mf, ns : ns + nl],
                                ps[:, :nl],
                                AF.Gelu,
                                bias=b1sb[:, mf : mf + 1],
                            )
                    # GEMM2 + bias + transpose + gate-scale + writeback + AllGather
                    for b in range(NHB if stage >= 4 else 0):
                        yst = pst.tile([128, NSL, HC], BF16, tag="ystage")
                        for mi in range(HC // 128):
                            mh = b * (HC // 128) + mi
                            w2t = pw2.tile([128, F // 128, 128], BF16, tag="w2")
                            eng = nc.sync if mh % 2 == 0 else nc.scalar
                            eng.dma_start(
                                out=w2t[:],
                                in_=W2b[mh].rearrange("p (kf h) -> p kf h", kf=F // 128),
                            )
                            for (ns, nl) in NCH:
                                ps = ppA.tile([128, 512], F32, tag="acc")
                                for kf in range(F // 128):
                                    nc.tensor.matmul(
                                        ps[:, :nl],
                                        w2t[:, kf, :],
                                        h1T[:, kf, ns : ns + nl],
                                        start=(kf == 0),
                                        stop=(kf == F // 128 - 1),
                                    )
                                yTs = pst.tile([128, 512], BF16, tag="yTs")
                                nc.vector.tensor_scalar(
                                    yTs[:, :nl], ps[:, :nl], b2sb[:, mh : mh + 1], None,
                                    op0=AO.add,
                                )
                                for sub in range(nl // 128):
                                    ps2 = ppB.tile([128, 128], BF16, tag="tp")
                                    nc.tensor.transpose(
                                        ps2[:], yTs[:, sub * 128 : (sub + 1) * 128], identb[:]
                                    )
                                    sc = ns // 128 + sub
                                    nc.vector.tensor_scalar(
                                        yst[:, sc, mi * 128 : (mi + 1) * 128],
                                        ps2[:],
                                        w10[:, sc : sc + 1],
                                        None,
                                        op0=AO.mult,
                                    )
                        nc.sync.dma_start(
                            out=y_w[b][0:CAP, :].rearrange("(v p) c -> p v c", p=128),
                            in_=yst[:],
                        )
                        nc.gpsimd.collective_compute(
                            "AllGather",
                            AO.bypass,
                            replica_groups=rg,
                            ins=[y_w[b][:, :]],
                            outs=[y_ag[b][:, :]],
                        )

            # ---------------- phase 6: owner combine ----------------
            with tc.tile_pool(name="comb", bufs=2) as pc:
                for b in range(NHB if stage >= 5 else 0):
                    yg = pc.tile([128, 16, HC], BF16, tag="yg")
                    for gc in range(2):
                        nc.gpsimd.dma_gather(
                            yg[:, gc * 8 : (gc + 1) * 8, :],
                            y_ag[b][:, :],
                            ownidx[:, gc * 64 : (gc + 1) * 64],
                            1024,
                            1024,
                            HC,
                            queue_num=_qn(b * 2 + gc),
                        )
                    osb = pc.tile([128, 8, HC], F32, tag="osb")
                    nc.vector.tensor_tensor(
                        osb[:], yg[:, 0:8, :], yg[:, 8:16, :], op=AO.add
                    )
                    nc.sync.dma_start(
                        out=out[:, b * HC : (b + 1) * HC].rearrange("(t p) c -> p t c", p=128),
                        in_=osb[:],
                    )

    if do_compile:
        nc.compile()
    return nc


_NC_CACHE = {}


def _get_nc(debug=False):
    stage = int(os.environ.get("KSTAGE", "5"))
    key = (bool(debug), stage)
    if key not in _NC_CACHE:
        _NC_CACHE[key] = build_nc(debug=debug, stage=stage)
    return _NC_CACHE[key]


def _in_maps(tokens, Wr, br, W1, b1, W2, b2):
    tokens = np.ascontiguousarray(np.asarray(tokens, np.float32))
    tokens_bf = np.ascontiguousarray(tokens.astype(BF_NP))
    Wr = np.ascontiguousarray(np.asarray(Wr, np.float32))
    br = np.ascontiguousarray(np.asarray(br, np.float32).reshape(1, E))
    maps = []
    for i in range(NC):
        W1i = np.asarray(W1[i], np.float32)
        W2i = np.asarray(W2[i], np.float32)
        # W1b[mf, p, kt*128+f] = W1[kt*128+p, mf*128+f]
        W1bi = np.ascontiguousarray(
            W1i.reshape(H // 128, 128, F // 128, 128)
            .transpose(2, 1, 0, 3)
            .reshape(F // 128, 128, H)
            .astype(BF_NP)
        )
        # W2b[mh, p, kf*128+h] = W2[kf*128+p, mh*128+h]
        W2bi = np.ascontiguousarray(
            W2i.reshape(F // 128, 128, H // 128, 128)
            .transpose(2, 1, 0, 3)
            .reshape(H // 128, 128, F)
            .astype(BF_NP)
        )
        maps.append(
            {
                "tok_sh": np.ascontiguousarray(tokens[i * TSH : (i + 1) * TSH]),
                "tokens_bf": tokens_bf,
                "Wr": Wr,
                "br": br,
                "W1b": W1bi,
                "b1s": np.ascontiguousarray(np.asarray(b1[i], np.float32).reshape(1, F)),
                "W2b": W2bi,
                "b2s": np.ascontiguousarray(np.asarray(b2[i], np.float32).reshape(1, H)),
            }
        )
    return maps


def kernel(tokens, Wr, br, W1, b1, W2, b2):
    global LAST_EXEC_NS
    debug = bool(int(os.environ.get("KDEBUG", "0")))
    trace = bool(int(os.environ.get("KTRACE", "0")))
    nc = _get_nc(debug=debug)
    in_maps = _in_maps(tokens, Wr, br, W1, b1, W2, b2)
    try:
        res = run_bass_kernel_spmd(nc, in_maps, list(range(NC)), trace=trace)
    except (ModuleNotFoundError, ImportError):
        res = run_bass_kernel_spmd(nc, in_maps, list(range(NC)), trace=False)
    LAST_EXEC_NS = res.exec_time_ns
    if debug:
        kernel.debug_results = res.results
    return np.concatenate([res.results[i]["out"] for i in range(NC)], axis=0)


def benchmark(tokens, Wr, br, W1, b1, W2, b2, iters=10):
    """Amortized per-call wall time of the compiled NEFF with device-resident
    inputs. Includes the PJRT/axon dispatch round-trip, so it is an upper
    bound on device execution time. Returns (amortized_ns, output)."""
    import time as _time

    import jax
    from jax.experimental.shard_map import shard_map
    from jax.sharding import Mesh, NamedSharding, PartitionSpec

    from concourse import bass2jax

    nc = _get_nc(debug=False)
    bass2jax.install_neuronx_cc_hook()
    partition_name = nc.partition_id_tensor.name if nc.partition_id_tensor else None
    in_names, out_names, out_avals, zero_outs = [], [], [], []
    for alloc in nc.m.functions[0].allocations:
        if not isinstance(alloc, mybir.MemoryLocationSet):
            continue
        name = alloc.memorylocations[0].name
        if alloc.kind == "ExternalInput":
            if name != partition_name:
                in_names.append(name)
        elif alloc.kind == "ExternalOutput":
            out_names.append(name)
            shape = tuple(alloc.tensor_shape)
            dtype = mybir.dt.np(alloc.dtype)
            out_avals.append(jax.core.ShapedArray(shape, dtype))
            zero_outs.append(np.zeros(shape, dtype))
    n_params, n_outs = len(in_names), len(out_avals)
    all_in = list(in_names) + out_names + ([partition_name] if partition_name else [])

    def _body(*args):
        operands = list(args)
        if partition_name is not None:
            operands.append(bass2jax.partition_id_tensor())
        return tuple(
            bass2jax._bass_exec_p.bind(
                *operands,
                out_avals=tuple(out_avals),
                in_names=tuple(all_in),
                out_names=tuple(out_names),
                lowering_input_output_aliases=(),
                sim_require_finite=True,
                sim_require_nnan=True,
                nc=nc,
            )
        )

    devices = jax.devices()[:NC]
    mesh = Mesh(np.asarray(devices), ("core",))
    sharded = jax.jit(
        shard_map(
            _body,
            mesh=mesh,
            in_specs=(PartitionSpec("core"),) * (n_params + n_outs),
            out_specs=(PartitionSpec("core"),) * n_outs,
            check_rep=False,
        ),
        donate_argnums=tuple(range(n_params, n_params + n_outs)),
        keep_unused=True,
    )
    maps = _in_maps(tokens, Wr, br, W1, b1, W2, b2)
    concat_in = [
        np.concatenate([maps[c][nm] for c in range(NC)], axis=0) for nm in in_names
    ]
    sharding = NamedSharding(mesh, PartitionSpec("core"))
    ins_dev = [jax.device_put(a, sharding) for a in concat_in]
    zeros_np = [np.zeros((NC * z.shape[0], *z.shape[1:]), z.dtype) for z in zero_outs]

    zs = [jax.device_put(z, sharding) for z in zeros_np]
    outs = sharded(*ins_dev, *zs)
    got = np.asarray(outs[out_names.index("out")]).reshape(N, H)

    staged = [[jax.device_put(z, sharding) for z in zeros_np] for _ in range(iters)]
    for z in staged:
        for a in z:
            a.block_until_ready()
    t0 = _time.perf_counter()
    rs = [sharded(*ins_dev, *staged[it]) for it in range(iters)]
    for r in rs:
        for a in r:
            a.block_until_ready()
    t1 = _time.perf_counter()
    return (t1 - t0) / iters * 1e9, got


# revision 12
# speedup vs baseline: 1.0337x; 1.0337x over previous
"""Trainium2 Bass kernel for AdaptiveTopKMoE (N=8192, H=2048, E=8, K=2, CF=1.25).

Expert-parallel across 8 NeuronCores: core i owns expert i's W1/W2 shard and
token shard [1024*i, 1024*(i+1)). Routing is computed replicated on every core
from an AllGathered logits tensor; capacity ranks come from exclusive prefix
sums done with triangular-ones matmuls; the per-expert compact table is built
with one-hot matmuls on the PE.

v2 layout: weights are pre-packed on the host into per-output-tile bf16 blocks
([128, contract*out] contiguous) so weight streaming runs on the HWDGE (sync/
scalar) engines as contiguous line-rate DMAs instead of 2048-descriptor
strided gpsimd DMAs. Tokens are also shipped pre-converted to bf16 so the FFN
x gather is a single transposed dma_gather straight into the [contract,
token] layout (no PE transposes / vector converts on the gather path). The
FFN runs in one 1280-slot pass (not two 640 halves), halving weight traffic.
The combine is 4 H-chunk AllGathers of gate-weighted expert outputs, each
overlapped with the next chunk's GEMM2, plus an owner-side dma_gather.
Invalid/dropped dispatches resolve to a zeroed trash row (slot 1280) so the
combine needs no masking.
"""

import os
import sys

for _p in ("/opt/trn_rl_repo", "/root/.axon_site/_ro/trn_rl_repo", "/root/.axon_site"):
    if os.path.isdir(_p) and _p not in sys.path:
        sys.path.insert(0, _p)

import numpy as np
import ml_dtypes

from concourse import bass, bacc, mybir, tile
from concourse.bass import ds
from concourse.bass_utils import run_bass_kernel_spmd

F32 = mybir.dt.float32
BF16 = mybir.dt.bfloat16
I32 = mybir.dt.int32
I16 = mybir.dt.int16
AF = mybir.ActivationFunctionType
AO = mybir.AluOpType
BF_NP = ml_dtypes.bfloat16

N, H, F, E, K = 8192, 2048, 4096, 8, 2
CAP = 1280            # ceil(1.25 * N / E)
NC = 8                # cores
TSH = N // NC         # 1024 tokens per core shard
RH = CAP + 1          # 1281 rows per core AG input (row 1280 = zero trash)
AGB = NC * RH         # 10248 rows in each AG output
HC = 512              # H-chunk per y AllGather
NHB = H // HC         # 4 H-chunks
NCH = [(0, 512), (512, 512), (1024, 256)]  # token n-chunks within the 1280 pass
NSL = CAP // 128      # 10 slot blocks of 128

LAST_EXEC_NS = None

# Sim-only escape hatch: CoreSim locks each SWDGE sem to one queue, so the
# round-robin queue_num choices below trip it. KSIMQ=1 forces queue 0.
_SINGLE_Q = bool(int(os.environ.get("KSIMQ", "0")))


def _qn(i):
    return 0 if _SINGLE_Q else i % 4


def build_nc(debug=False, do_compile=True, stage=5):
    nc = bacc.Bacc(
        "TRN2", target_bir_lowering=False, debug=False, num_devices=NC,
        num_swdge_queues=4,
    )

    tok_sh = nc.dram_tensor("tok_sh", [TSH, H], F32, kind="ExternalInput").ap()
    tokens_bf = nc.dram_tensor("tokens_bf", [N, H], BF16, kind="ExternalInput").ap()
    Wr = nc.dram_tensor("Wr", [E, H], F32, kind="ExternalInput").ap()
    br = nc.dram_tensor("br", [1, E], F32, kind="ExternalInput").ap()
    W1b = nc.dram_tensor("W1b", [F // 128, 128, H], BF16, kind="ExternalInput").ap()
    b1s = nc.dram_tensor("b1s", [1, F], F32, kind="ExternalInput").ap()
    W2b = nc.dram_tensor("W2b", [H // 128, 128, F], BF16, kind="ExternalInput").ap()
    b2s = nc.dram_tensor("b2s", [1, H], F32, kind="ExternalInput").ap()
    out = nc.dram_tensor("out", [TSH, H], F32, kind="ExternalOutput").ap()

    lg_slice = nc.dram_tensor("lg_slice", [TSH, E], F32).ap()
    lg_full = nc.dram_tensor("lg_full", [N, E], F32, addr_space="Shared").ap()
    g_lin = nc.dram_tensor("g_lin", [2 * N], F32).ap()
    tok_dram = nc.dram_tensor("tok_dram", [CAP], I16).ap()
    # y_w[hc]: this core's gate-weighted expert output rows for one H-chunk
    y_w = [nc.dram_tensor(f"y_w{b}", [RH, HC], BF16).ap() for b in range(NHB)]
    # y_ag[hc]: AllGathered blocks, row e*RH + s
    y_ag = [
        nc.dram_tensor(f"y_ag{b}", [AGB, HC], BF16, addr_space="Shared").ap()
        for b in range(NHB)
    ]
    if debug:
        dbg_rk = nc.dram_tensor("dbg_rk", [4, N], F32, kind="ExternalOutput").ap()

    rg = [list(range(NC))]

    with tile.TileContext(nc) as tc:
        with (
            tc.tile_pool(name="main", bufs=1) as pm,
            tc.tile_pool(name="ppA", bufs=3, space="PSUM") as ppA,
            tc.tile_pool(name="ppB", bufs=2, space="PSUM") as ppB,
        ):
            # ---------------- phase 0: constants ----------------
            pid_gp = nc.gpsimd.partition_id()

            iotaP = pm.tile([128, 1], I32, tag="iotaP")
            nc.gpsimd.iota(iotaP[:], pattern=[[0, 1]], base=0, channel_multiplier=1)
            pcol = pm.tile([128, 1], F32, tag="pcol")
            nc.vector.tensor_copy(pcol[:], iotaP[:])

            iotaF = pm.tile([128, 128], I32, tag="iotaF")
            nc.gpsimd.iota(iotaF[:], pattern=[[1, 128]], base=0, channel_multiplier=0)
            frow = pm.tile([128, 128], F32, tag="frow")
            nc.vector.tensor_copy(frow[:], iotaF[:])

            U = pm.tile([128, 128], F32, tag="U")
            nc.vector.tensor_scalar(U[:], frow[:], pcol[:], None, op0=AO.is_gt)
            ident = pm.tile([128, 128], F32, tag="ident")
            nc.vector.tensor_scalar(ident[:], frow[:], pcol[:], None, op0=AO.is_equal)
            identb = pm.tile([128, 128], BF16, tag="identb")
            nc.vector.tensor_copy(identb[:], ident[:])

            iotaE3 = pm.tile([128, 64, 8], I32, tag="iotaE3")
            nc.gpsimd.iota(iotaE3[:], pattern=[[0, 64], [1, 8]], base=0, channel_multiplier=0)
            iotaE3f = pm.tile([128, 64, 8], F32, tag="iotaE3f")
            nc.vector.tensor_copy(iotaE3f[:], iotaE3[:])

            tio = pm.tile([128, 64], I32, tag="tio")
            nc.gpsimd.iota(tio[:], pattern=[[128, 64]], base=0, channel_multiplier=1)
            tiof = pm.tile([128, 64], F32, tag="tiof")
            nc.vector.tensor_copy(tiof[:], tio[:])

            b1sb = pm.tile([128, F // 128], F32, tag="b1sb")
            nc.sync.dma_start(out=b1sb[:], in_=b1s.rearrange("o (m p) -> p (o m)", p=128))
            b2sb = pm.tile([128, H // 128], F32, tag="b2sb")
            nc.sync.dma_start(out=b2sb[:], in_=b2s.rearrange("o (m p) -> p (o m)", p=128))

            WrT = pm.tile([128, H // 128, E], F32, tag="WrT")
            WrT_src = Wr.rearrange("e (kt p) -> p kt e", p=128)
            for kt in range(H // 128):
                nc.sync.dma_start(out=WrT[:, kt, :], in_=WrT_src[:, kt, :])

            brsb = pm.tile([1, E], F32, tag="brsb")
            nc.sync.dma_start(out=brsb[:], in_=br[:, :])
            brb = pm.tile([128, E], F32, tag="brb")
            nc.gpsimd.partition_broadcast(brb[:], brsb[:])

            pidt = pm.tile([1, 1], I32, tag="pidt")
            nc.gpsimd.reg_save(pidt[0:1, 0:1], pid_gp)
            pidb = pm.tile([128, 1], I32, tag="pidb")
            nc.gpsimd.partition_broadcast(pidb[:], pidt[:])
            pidf = pm.tile([128, 1], F32, tag="pidf")
            nc.vector.tensor_copy(pidf[:], pidb[:])

            zrow = pm.tile([1, HC], BF16, tag="zrow")
            nc.vector.memset(zrow[:], 0.0)
            # trash rows of the 4 AG inputs: write once, up front
            for b in range(NHB):
                nc.scalar.dma_start(out=y_w[b][CAP : CAP + 1, :], in_=zrow[:])
            # one-hot table-build constants: s128b[p, tau, S] = S*128
            s128b = pm.tile([128, 64, NSL], F32, tag="s128b")
            s128i = pm.tile([128, 64, NSL], I32, tag="s128i")
            nc.gpsimd.iota(s128i[:], pattern=[[0, 64], [128, NSL]], base=0, channel_multiplier=0)
            nc.vector.tensor_copy(s128b[:], s128i[:])
            w10 = pm.tile([128, NSL], F32, tag="w10")

            tokidx = pm.tile([128, CAP // 16], I16, tag="tokidx")
            ownidx = pm.tile([128, 2 * TSH // 16], I16, tag="ownidx")

            # ---------------- phases 1-2: router + routing + tables ----------
            with tc.tile_pool(name="route", bufs=1) as pr, tc.tile_pool(
                name="rtok", bufs=3
            ) as prt:
                for tt in range(TSH // 128):
                    tsl = prt.tile([128, H], F32, tag="tsl")
                    teng = nc.sync if tt % 2 == 0 else nc.scalar
                    teng.dma_start(out=tsl[:], in_=tok_sh[tt * 128 : (tt + 1) * 128, :])
                    xTr = prt.tile([128, H // 128, 128], F32, tag="xTr")
                    for kt in range(H // 128):
                        ps = ppB.tile([128, 128], F32, tag="tp")
                        nc.tensor.transpose(ps[:], tsl[:, kt * 128 : (kt + 1) * 128], ident[:])
                        # split PSUM->SBUF copies across scalar+vector so the
                        # router head isn't serialized on one engine
                        if kt % 2 == 0:
                            nc.scalar.activation(xTr[:, kt, :], ps[:], AF.Copy)
                        else:
                            nc.vector.tensor_copy(xTr[:, kt, :], ps[:])
                    plg = ppB.tile([128, E], F32, tag="plg")
                    for kt in range(H // 128):
                        nc.tensor.matmul(
                            plg[:], xTr[:, kt, :], WrT[:, kt, :],
                            start=(kt == 0), stop=(kt == H // 128 - 1),
                        )
                    lgs = pr.tile([128, E], F32, tag="lgs")
                    nc.vector.tensor_tensor(lgs[:], plg[:], brb[:], op=AO.add)
                    nc.sync.dma_start(out=lg_slice[tt * 128 : (tt + 1) * 128, :], in_=lgs[:])

                nc.gpsimd.collective_compute(
                    "AllGather", AO.bypass, replica_groups=rg,
                    ins=[lg_slice[:, :]], outs=[lg_full[:, :]],
                )

                lg = pr.tile([128, 64, 8], F32, tag="lg")
                nc.sync.dma_start(out=lg[:], in_=lg_full.rearrange("(tau p) e -> p tau e", p=128))
                m1 = pr.tile([128, 64], F32, tag="m1")
                nc.vector.tensor_reduce(m1[:], lg[:], axis=mybir.AxisListType.X, op=AO.max)
                eq1 = pr.tile([128, 64, 8], F32, tag="eq1")
                nc.vector.tensor_tensor(
                    eq1[:], lg[:], m1[:].to_broadcast((128, 64, 8)), op=AO.is_equal
                )
                tmp3 = pr.tile([128, 64, 8], F32, tag="tmp3")
                nc.vector.tensor_tensor(tmp3[:], eq1[:], iotaE3f[:], op=AO.mult)
                idx1 = pr.tile([128, 64], F32, tag="idx1")
                nc.vector.tensor_reduce(idx1[:], tmp3[:], axis=mybir.AxisListType.X, op=AO.add)
                masked = pr.tile([128, 64, 8], F32, tag="masked")
                nc.vector.scalar_tensor_tensor(
                    masked[:], eq1[:], -1e30, lg[:], op0=AO.mult, op1=AO.add
                )
                m2 = pr.tile([128, 64], F32, tag="m2")
                nc.vector.tensor_reduce(m2[:], masked[:], axis=mybir.AxisListType.X, op=AO.max)
                eq2 = pr.tile([128, 64, 8], F32, tag="eq2")
                nc.vector.tensor_tensor(
                    eq2[:], masked[:], m2[:].to_broadcast((128, 64, 8)), op=AO.is_equal
                )
                nc.vector.tensor_tensor(tmp3[:], eq2[:], iotaE3f[:], op=AO.mult)
                idx2 = pr.tile([128, 64], F32, tag="idx2")
                nc.vector.tensor_reduce(idx2[:], tmp3[:], axis=mybir.AxisListType.X, op=AO.add)
                dd = pr.tile([128, 64], F32, tag="dd")
                nc.vector.tensor_tensor(dd[:], m2[:], m1[:], op=AO.subtract)
                ex = pr.tile([128, 64], F32, tag="ex")
                nc.scalar.activation(ex[:], dd[:], AF.Exp)
                exp1 = pr.tile([128, 64], F32, tag="exp1")
                nc.vector.tensor_scalar_add(exp1[:], ex[:], 1.0)
                p1 = pr.tile([128, 64], F32, tag="p1")
                nc.vector.reciprocal(p1[:], exp1[:])
                p2 = pr.tile([128, 64], F32, tag="p2")
                nc.vector.tensor_scalar(p2[:], p1[:], -1.0, 1.0, op0=AO.mult, op1=AO.add)

                C = pr.tile([128, 64, 8], F32, tag="C")
                nc.vector.tensor_tensor(C[:], eq1[:], eq2[:], op=AO.add)
                C512 = C[:].rearrange("p a e -> p (a e)")
                Pin = ppA.tile([128, 512], F32, tag="acc")
                nc.tensor.matmul(Pin[:], U[:], C512, start=True, stop=True)
                ones1 = pr.tile([128, 1], F32, tag="ones1")
                nc.vector.memset(ones1[:], 1.0)
                Tps = ppA.tile([128, 512], F32, tag="acc")
                nc.tensor.matmul(Tps[0:1, :], ones1[:], C512, start=True, stop=True)
                Trow = pr.tile([1, 512], F32, tag="Trow")
                nc.scalar.activation(Trow[:], Tps[0:1, :], AF.Copy)
                Tcol = pr.tile([64, 8], F32, tag="Tcol")
                nc.sync.dma_start(out=Tcol[:], in_=Trow[:])
                cps = ppB.tile([64, 8], F32, tag="plg")
                nc.tensor.matmul(cps[:], U[0:64, 0:64], Tcol[:], start=True, stop=True)
                csb = pr.tile([64, 8], F32, tag="csb")
                nc.scalar.activation(csb[:], cps[:], AF.Copy)
                crow = pr.tile([1, 512], F32, tag="crow")
                nc.sync.dma_start(out=crow[:], in_=csb[:])
                cb = pr.tile([128, 512], F32, tag="cb")
                nc.gpsimd.partition_broadcast(cb[:], crow[:])
                rkf = pr.tile([128, 64, 8], F32, tag="rkf")
                nc.vector.tensor_tensor(
                    rkf[:].rearrange("p a e -> p (a e)"), Pin[:], cb[:], op=AO.add
                )
                rank1 = pr.tile([128, 64], F32, tag="rank1")
                nc.vector.tensor_tensor(tmp3[:], rkf[:], eq1[:], op=AO.mult)
                nc.vector.tensor_reduce(rank1[:], tmp3[:], axis=mybir.AxisListType.X, op=AO.add)
                rank2 = pr.tile([128, 64], F32, tag="rank2")
                nc.vector.tensor_tensor(tmp3[:], rkf[:], eq2[:], op=AO.mult)
                nc.vector.tensor_reduce(rank2[:], tmp3[:], axis=mybir.AxisListType.X, op=AO.add)
                valid1 = pr.tile([128, 64], F32, tag="valid1")
                nc.vector.tensor_scalar(valid1[:], rank1[:], float(CAP), None, op0=AO.is_lt)
                valid2 = pr.tile([128, 64], F32, tag="valid2")
                nc.vector.tensor_scalar(valid2[:], rank2[:], float(CAP), None, op0=AO.is_lt)
                w1t_ = pr.tile([128, 64], F32, tag="w1t_")
                nc.vector.tensor_tensor(w1t_[:], p1[:], valid1[:], op=AO.mult)
                w2t_ = pr.tile([128, 64], F32, tag="w2t_")
                nc.vector.tensor_tensor(w2t_[:], p2[:], valid2[:], op=AO.mult)
                # rp_k = valid ? rank : CAP
                rp1 = pr.tile([128, 64], F32, tag="rp1")
                nc.vector.scalar_tensor_tensor(
                    rp1[:], rank1[:], -float(CAP), valid1[:], op0=AO.add, op1=AO.mult
                )
                nc.vector.tensor_scalar_add(rp1[:], rp1[:], float(CAP))
                rp2 = pr.tile([128, 64], F32, tag="rp2")
                nc.vector.scalar_tensor_tensor(
                    rp2[:], rank2[:], -float(CAP), valid2[:], op0=AO.add, op1=AO.mult
                )
                nc.vector.tensor_scalar_add(rp2[:], rp2[:], float(CAP))
                # AG row ids: g = idx*RH + rp; invalid (rp = CAP) lands on the
                # expert's trash row automatically.
                g1 = pr.tile([128, 64], F32, tag="g1")
                g2 = pr.tile([128, 64], F32, tag="g2")
                for (gk, idxk, rpk) in ((g1, idx1, rp1), (g2, idx2, rp2)):
                    nc.vector.scalar_tensor_tensor(
                        gk[:], idxk[:], float(RH), rpk[:], op0=AO.mult, op1=AO.add
                    )
                if debug:
                    dview = dbg_rk.rearrange("k (tau p) -> k p tau", p=128)
                    nc.sync.dma_start(out=dview[0], in_=rank1[:])
                    nc.sync.dma_start(out=dview[1], in_=rank2[:])
                    nc.sync.dma_start(out=dview[2], in_=g1[:])
                    nc.sync.dma_start(out=dview[3], in_=g2[:])

                # expert-side select for my expert
                m1e = pr.tile([128, 64], F32, tag="m1e")
                nc.vector.tensor_scalar(m1e[:], idx1[:], pidf[:], None, op0=AO.is_equal)
                m2e = pr.tile([128, 64], F32, tag="m2e")
                nc.vector.tensor_scalar(m2e[:], idx2[:], pidf[:], None, op0=AO.is_equal)
                w_e = pr.tile([128, 64], F32, tag="w_e")
                nc.vector.tensor_tensor(w_e[:], w1t_[:], m1e[:], op=AO.mult)
                tmpb = pr.tile([128, 64], F32, tag="tmpb")
                nc.vector.tensor_tensor(tmpb[:], w2t_[:], m2e[:], op=AO.mult)
                nc.vector.tensor_tensor(w_e[:], w_e[:], tmpb[:], op=AO.add)
                # local slot: m1e*(rp1-CAP) + m2e*(rp2-CAP) + CAP
                sl = pr.tile([128, 64], F32, tag="sl")
                nc.vector.tensor_scalar_add(tmpb[:], rp1[:], -float(CAP))
                nc.vector.tensor_tensor(sl[:], tmpb[:], m1e[:], op=AO.mult)
                nc.vector.tensor_scalar_add(tmpb[:], rp2[:], -float(CAP))
                nc.vector.tensor_tensor(tmpb[:], tmpb[:], m2e[:], op=AO.mult)
                nc.vector.tensor_tensor(sl[:], sl[:], tmpb[:], op=AO.add)
                nc.vector.tensor_scalar_add(sl[:], sl[:], float(CAP))

                if stage >= 2:
                    # table build via one-hot matmuls on the (idle) PE:
                    # slot = S*128 + r; psum[r, (S, c)] += 1{r(t)=r} * qm_S(t) * val_c(t)
                    # accumulated over the 64 token-chunks. Invalid slots
                    # (sl=CAP=1280) have qm=0 for all S -> contribute nothing.
                    # t1 = S*128 - sl (bulk), qm = 1{-128 < t1 <= 0}
                    t1b = pr.tile([128, 64, NSL], F32, tag="t1b")
                    nc.vector.tensor_tensor(
                        t1b[:], s128b[:], sl[:].to_broadcast((128, 64, NSL)),
                        op=AO.subtract,
                    )
                    qma = pr.tile([128, 64, NSL], F32, tag="qma")
                    nc.vector.tensor_scalar(qma[:], t1b[:], -128.0, None, op0=AO.is_gt)
                    qmb = pr.tile([128, 64, NSL], F32, tag="qmb")
                    nc.vector.tensor_scalar(qmb[:], t1b[:], 0.5, None, op0=AO.is_lt)
                    nc.vector.tensor_tensor(qma[:], qma[:], qmb[:], op=AO.mult)
                    # r = sl mod 128 = -sum_S qm*t1
                    nc.vector.tensor_tensor(qmb[:], qma[:], t1b[:], op=AO.mult)
                    rmod = pr.tile([128, 64], F32, tag="rmod")
                    nc.vector.tensor_reduce(rmod[:], qmb[:], axis=mybir.AxisListType.X, op=AO.add)
                    nc.vector.tensor_scalar(rmod[:], rmod[:], -1.0, None, op0=AO.mult)

                    # bulk-precompute both one-hot payload planes in 2 DVE ops
                    # (instead of 128 small ops inside the tau loop)
                    valqAB = pr.tile([128, 64, 2, NSL], F32, tag="valqAB")
                    nc.vector.tensor_tensor(
                        valqAB[:, :, 0, :], qma[:], tiof[:].to_broadcast((128, 64, NSL)),
                        op=AO.mult,
                    )
                    nc.vector.tensor_tensor(
                        valqAB[:, :, 1, :], qma[:], w_e[:].to_broadcast((128, 64, NSL)),
                        op=AO.mult,
                    )
                    with tc.tile_pool(name="ptab", bufs=1, space="PSUM") as ppt, \
                         tc.tile_pool(name="onehot", bufs=3) as poh:
                        pstab = ppt.tile([128, 2, NSL], F32, tag="ptab")
                        pstab2 = pstab[:].rearrange("p c a -> p (c a)")
                        for tau in range(64):
                            R = poh.tile([128, 128], F32, tag="R")
                            nc.vector.tensor_scalar(
                                R[:], frow[:], rmod[:, tau : tau + 1], None,
                                op0=AO.is_equal,
                            )
                            nc.tensor.matmul(
                                pstab2, R[:],
                                valqAB[:, tau, :, :].rearrange("p c a -> p (c a)"),
                                start=(tau == 0), stop=(tau == 63),
                            )
                        # weights stay on-chip in FFN layout [p, S]
                        nc.scalar.activation(w10[:], pstab[:, 1, :], AF.Copy)
                        # token ids -> DRAM (slot order) -> 16-wrap -> doublings
                        toki = pr.tile([128, NSL], I16, tag="toki")
                        nc.vector.tensor_copy(toki[:], pstab[:, 0, :])
                    nc.sync.dma_start(
                        out=tok_dram.rearrange("(s p) -> p s", p=128), in_=toki[:]
                    )
                    nc.sync.dma_start(
                        out=tokidx[0:16, :],
                        in_=tok_dram.rearrange("(f part) -> part f", part=16),
                    )
                    nc.scalar.dma_start(out=tokidx[16:32, :], in_=tokidx[0:16, :])
                    nc.gpsimd.dma_start(out=tokidx[32:64, :], in_=tokidx[0:32, :])
                    nc.sync.dma_start(out=tokidx[64:128, :], in_=tokidx[0:64, :])

                    # owner-side gather ids j = k*TSH + t_local, wrapped in 16
                    nc.sync.dma_start(out=g_lin[0:N].rearrange("(tau p) -> p tau", p=128), in_=g1[:])
                    nc.sync.dma_start(
                        out=g_lin[N : 2 * N].rearrange("(tau p) -> p tau", p=128), in_=g2[:]
                    )
                    ownf = pr.tile([16, 2 * TSH // 16], F32, tag="ownf")
                    gview = g_lin.rearrange("(k o f part) -> part k o f", part=16, k=2, o=8)
                    pid_sy = nc.sync.partition_id()
                    pid_sc = nc.scalar.partition_id()
                    for k, (eng, pid_e) in enumerate(
                        ((nc.sync, pid_sy), (nc.scalar, pid_sc))
                    ):
                        eng.dma_start(
                            out=ownf[:, k * 64 : (k + 1) * 64],
                            in_=gview[:, k, ds(pid_e, 1), :],
                        )
                    nc.vector.tensor_copy(ownidx[0:16, :], ownf[:])
                    nc.scalar.dma_start(out=ownidx[16:32, :], in_=ownidx[0:16, :])
                    nc.gpsimd.dma_start(out=ownidx[32:64, :], in_=ownidx[0:32, :])
                    nc.sync.dma_start(out=ownidx[64:128, :], in_=ownidx[0:64, :])

            # ---------------- phases 3-5: gather, GEMM1, GEMM2+AG ----------
            if stage >= 3:
                with (
                    tc.tile_pool(name="ffn", bufs=1) as pf,
                    tc.tile_pool(name="w1p", bufs=3) as pw1,
                    tc.tile_pool(name="w2p", bufs=2) as pw2,
                    tc.tile_pool(name="stg", bufs=2) as pst,
                ):
                    # chunked transposed gathers (SWDGE descriptor FIFO holds
                    # 128 entries; a 1280-idx transposed gather needs 259):
                    # xTc[ci][p, kt, s] = tokens_bf[tok[ns+s], kt*128+p]
                    xT0 = pf.tile([128, H // 128, NCH[0][1]], BF16, tag="xT0")
                    xT1 = pf.tile([128, H // 128, NCH[1][1]], BF16, tag="xT1")
                    xT2 = pf.tile([128, H // 128, NCH[2][1]], BF16, tag="xT2")
                    xTc = [xT0, xT1, xT2]
                    h1T = pf.tile([128, F // 128, CAP], BF16, tag="h1T")
                    for ci, (ns, nl) in enumerate(NCH):
                        nc.gpsimd.dma_gather(
                            xTc[ci][:, :, :],
                            tokens_bf[:, :],
                            tokidx[:, ns // 16 : (ns + nl) // 16],
                            nl,
                            nl,
                            H,
                            transpose=True,
                            queue_num=_qn(ci),
                        )
                    # GEMM1 + gelu
                    for mf in range(F // 128):
                        w1t = pw1.tile([128, H // 128, 128], BF16, tag="w1")
                        eng = nc.sync if mf % 2 == 0 else nc.scalar
                        eng.dma_start(
                            out=w1t[:],
                            in_=W1b[mf].rearrange("p (kt f) -> p kt f", kt=H // 128),
                        )
                        for ci, (ns, nl) in enumerate(NCH):
                            ps = ppA.tile([128, 512], F32, tag="acc")
                            for kt in range(H // 128):
                                nc.tensor.matmul(
                                    ps[:, :nl],
                                    w1t[:, kt, :],
                                    xTc[ci][:, kt, :],
                                    start=(kt == 0),
                                    stop=(kt == H // 128 - 1),
                                )
                            nc.scalar.activation(
                                h1T[:, mf, ns : ns + nl],
                                ps[:, :nl],
                                AF.Gelu,
                                bias=b1sb[:, mf : mf + 1],
                            )
                    # GEMM2 + bias + transpose + gate-scale + writeback + AllGather
                    for b in range(NHB if stage >= 4 else 0):
                        yst = pst.tile([128, NSL, HC], BF16, tag="ystage")
                        for mi in range(HC // 128):
                            mh = b * (HC // 128) + mi
                            w2t = pw2.tile([128, F // 128, 128], BF16, tag="w2")
                            eng = nc.sync if mh % 2 == 0 else nc.scalar
                            eng.dma_start(
                                out=w2t[:],
                                in_=W2b[mh].rearrange("p (kf h) -> p kf h", kf=F // 128),
                            )
                            for (ns, nl) in NCH:
                                ps = ppA.tile([128, 512], F32, tag="acc")
                                for kf in range(F // 128):
                                    nc.tensor.matmul(
                                        ps[:, :nl],
                                        w2t[:, kf, :],
                                        h1T[:, kf, ns : ns + nl],
                                        start=(kf == 0),
                                        stop=(kf == F // 128 - 1),
                                    )
                                yTs = pst.tile([128, 512], BF16, tag="yTs")
                                nc.vector.tensor_scalar(
                                    yTs[:, :nl], ps[:, :nl], b2sb[:, mh : mh + 1], None,
                                    op0=AO.add,
                                )
                                for sub in range(nl // 128):
                                    ps2 = ppB.tile([128, 128], BF16, tag="tp")
                                    nc.tensor.transpose(
                                        ps2[:], yTs[:, sub * 128 : (sub + 1) * 128], identb[:]
                                    )
                                    sc = ns // 128 + sub
                                    nc.vector.tensor_scalar(
                                        yst[:, sc, mi * 128 : (mi + 1) * 128],
                                        ps2[:],
                                        w10[:, sc : sc + 1],
                                        None,
                                        op0=AO.mult,
                                    )
                        nc.sync.dma_start(
                            out=y_w[b][0:CAP, :].rearrange("(v p) c -> p v c", p=128),
                            in_=yst[:],
                        )
                        nc.gpsimd.collective_compute(
                            "AllGather",
                            AO.bypass,
                            replica_groups=rg,
                            ins=[y_w[b][:, :]],
                            outs=[y_ag[b][:, :]],
                        )

            # ---------------- phase 6: owner combine ----------------
            with tc.tile_pool(name="comb", bufs=2) as pc:
                for b in range(NHB if stage >= 5 else 0):
                    yg = pc.tile([128, 16, HC], BF16, tag="yg")
                    for gc in range(2):
                        nc.gpsimd.dma_gather(
                            yg[:, gc * 8 : (gc + 1) * 8, :],
                            y_ag[b][:, :],
                            ownidx[:, gc * 64 : (gc + 1) * 64],
                            1024,
                            1024,
                            HC,
                            queue_num=_qn(b * 2 + gc),
                        )
                    osb = pc.tile([128, 8, HC], F32, tag="osb")
                    nc.vector.tensor_tensor(
                        osb[:], yg[:, 0:8, :], yg[:, 8:16, :], op=AO.add
                    )
                    nc.sync.dma_start(
                        out=out[:, b * HC : (b + 1) * HC].rearrange("(t p) c -> p t c", p=128),
                        in_=osb[:],
                    )

    if do_compile:
        nc.compile()
    return nc


_NC_CACHE = {}


def _get_nc(debug=False):
    stage = int(os.environ.get("KSTAGE", "5"))
    key = (bool(debug), stage)
    if key not in _NC_CACHE:
        _NC_CACHE[key] = build_nc(debug=debug, stage=stage)
    return _NC_CACHE[key]


def _in_maps(tokens, Wr, br, W1, b1, W2, b2):
    tokens = np.ascontiguousarray(np.asarray(tokens, np.float32))
    tokens_bf = np.ascontiguousarray(tokens.astype(BF_NP))
    Wr = np.ascontiguousarray(np.asarray(Wr, np.float32))
    br = np.ascontiguousarray(np.asarray(br, np.float32).reshape(1, E))
    maps = []
    for i in range(NC):
        W1i = np.asarray(W1[i], np.float32)
        W2i = np.asarray(W2[i], np.float32)
        # W1b[mf, p, kt*128+f] = W1[kt*128+p, mf*128+f]
        W1bi = np.ascontiguousarray(
            W1i.reshape(H // 128, 128, F // 128, 128)
            .transpose(2, 1, 0, 3)
            .reshape(F // 128, 128, H)
            .astype(BF_NP)
        )
        # W2b[mh, p, kf*128+h] = W2[kf*128+p, mh*128+h]
        W2bi = np.ascontiguousarray(
            W2i.reshape(F // 128, 128, H // 128, 128)
            .transpose(2, 1, 0, 3)
            .reshape(H // 128, 128, F)
            .astype(BF_NP)
        )
        maps.append(
            {
                "tok_sh": np.ascontiguousarray(tokens[i * TSH : (i + 1) * TSH]),
                "tokens_bf": tokens_bf,
                "Wr": Wr,
                "br": br,
                "W1b": W1bi,
                "b1s": np.ascontiguousarray(np.asarray(b1[i], np.float32).reshape(1, F)),
                "W2b": W2bi,
                "b2s": np.ascontiguousarray(np.asarray(b2[i], np.float32).reshape(1, H)),
            }
        )
    return maps


def kernel(tokens, Wr, br, W1, b1, W2, b2):
    global LAST_EXEC_NS
    debug = bool(int(os.environ.get("KDEBUG", "0")))
    trace = bool(int(os.environ.get("KTRACE", "0")))
    nc = _get_nc(debug=debug)
    in_maps = _in_maps(tokens, Wr, br, W1, b1, W2, b2)
    try:
        res = run_bass_kernel_spmd(nc, in_maps, list(range(NC)), trace=trace)
    except (ModuleNotFoundError, ImportError):
        res = run_bass_kernel_spmd(nc, in_maps, list(range(NC)), trace=False)
    LAST_EXEC_NS = res.exec_time_ns
    if debug:
        kernel.debug_results = res.results
    return np.concatenate([res.results[i]["out"] for i in range(NC)], axis=0)


def benchmark(tokens, Wr, br, W1, b1, W2, b2, iters=25):
    """Amortized per-call wall time of the compiled NEFF with device-resident
    inputs. Includes the PJRT/axon dispatch round-trip, so it is an upper
    bound on device execution time. Returns (amortized_ns, output)."""
    import time as _time

    import jax
    from jax.experimental.shard_map import shard_map
    from jax.sharding import Mesh, NamedSharding, PartitionSpec

    from concourse import bass2jax

    nc = _get_nc(debug=False)
    bass2jax.install_neuronx_cc_hook()
    partition_name = nc.partition_id_tensor.name if nc.partition_id_tensor else None
    in_names, out_names, out_avals, zero_outs = [], [], [], []
    for alloc in nc.m.functions[0].allocations:
        if not isinstance(alloc, mybir.MemoryLocationSet):
            continue
        name = alloc.memorylocations[0].name
        if alloc.kind == "ExternalInput":
            if name != partition_name:
                in_names.append(name)
        elif alloc.kind == "ExternalOutput":
            out_names.append(name)
            shape = tuple(alloc.tensor_shape)
            dtype = mybir.dt.np(alloc.dtype)
            out_avals.append(jax.core.ShapedArray(shape, dtype))
            zero_outs.append(np.zeros(shape, dtype))
    n_params, n_outs = len(in_names), len(out_avals)
    all_in = list(in_names) + out_names + ([partition_name] if partition_name else [])

    def _body(*args):
        operands = list(args)
        if partition_name is not None:
            operands.append(bass2jax.partition_id_tensor())
        return tuple(
            bass2jax._bass_exec_p.bind(
                *operands,
                out_avals=tuple(out_avals),
                in_names=tuple(all_in),
                out_names=tuple(out_names),
                lowering_input_output_aliases=(),
                sim_require_finite=True,
                sim_require_nnan=True,
                nc=nc,
            )
        )

    devices = jax.devices()[:NC]
    mesh = Mesh(np.asarray(devices), ("core",))
    sharded = jax.jit(
        shard_map(
            _body,
            mesh=mesh,
            in_specs=(PartitionSpec("core"),) * (n_params + n_outs),
            out_specs=(PartitionSpec("core"),) * n_outs,
            check_rep=False,
        ),
        donate_argnums=tuple(range(n_params, n_params + n_outs)),
        keep_unused=True,
    )
    maps = _in_maps(tokens, Wr, br, W1, b1, W2, b2)
    concat_in = [
        np.concatenate([maps[c][nm] for c in range(NC)], axis=0) for nm in in_names
    ]
    sharding = NamedSharding(mesh, PartitionSpec("core"))
    ins_dev = [jax.device_put(a, sharding) for a in concat_in]
    zeros_np = [np.zeros((NC * z.shape[0], *z.shape[1:]), z.dtype) for z in zero_outs]

    zs = [jax.device_put(z, sharding) for z in zeros_np]
    outs = sharded(*ins_dev, *zs)
    got = np.asarray(outs[out_names.index("out")]).reshape(N, H)

    staged = [[jax.device_put(z, sharding) for z in zeros_np] for _ in range(iters)]
    for z in staged:
        for a in z:
            a.block_until_ready()
    t0 = _time.perf_counter()
    rs = [sharded(*ins_dev, *staged[it]) for it in range(iters)]
    for r in rs:
        for a in r:
            a.block_until_ready()
    t1 = _time.perf_counter()
    return (t1 - t0) / iters * 1e9, got
